# revision 19
# baseline (speedup 1.0000x reference)
"""JointMultiHeadedAttention TRN2 kernel.

Sharding: 8 cores = batch(2) x head-group(4). Core k handles batch b=k//4,
heads 4g..4g+3 where g=k%4 (feature cols 256g..256g+256).

Per-core device program (all matmuls fp32r):
  A: DMA X=[1024,1024], C=[512,1024]; PE-transpose -> XT [f,tok], CT.
  B: projections -> QT [d,q] (2 tiles [128,1024]), KT [d,k] (2 x [128,1536]),
     V [k,d] ([128,12,256]); biases via ACT-evac (Q/K) and ones-row matmul (V).
  C: per (qgroup, head, qtile): scores = QT.T@KT slices -> psum [128,1536];
     DVE evac+mask-add; ACT exp (accum row sums); DVE normalize -> attn;
     DMA attn out; PE-transpose attn -> attnT [k,q]; context matmul
     lhsT=V-chunk rhs=attnT -> ctxT [d,q]; h==0: top_attn.
  D: outproj: lhsT=ctxT chunks, rhs=WoT -> partial out [1024,1024]; DMA.
Host: slice/transpose weights, assemble outputs, sum partials + bias.
"""
import math
from contextlib import ExitStack

import numpy as np

import concourse.bass as bass
import concourse.mybir as mybir
import concourse.tile as tile
from concourse import bacc
from concourse.bass_utils import run_bass_kernel_spmd
from concourse.masks import make_identity

F32 = mybir.dt.float32
F32R = mybir.dt.float32r
U8 = mybir.dt.uint8
AF = mybir.ActivationFunctionType

H, DM, D = 16, 1024, 64
B, SL, CL = 2, 1024, 512
KL = SL + CL          # 1536
HG = 4                # heads per core
DG = HG * D           # 256 feature cols per core
NEG = -1e30

_CACHE = {}


def _build():
    nc = bacc.Bacc("TRN2", target_bir_lowering=False, debug=False, num_devices=8)

    x_d = nc.dram_tensor("x", [SL, DM], F32, kind="ExternalInput")
    c_d = nc.dram_tensor("c", [CL, DM], F32, kind="ExternalInput")
    msk_d = nc.dram_tensor("msk", [SL, SL], U8, kind="ExternalInput")
    mctx_d = nc.dram_tensor("mctx", [128, CL], F32, kind="ExternalInput")
    wq_d = nc.dram_tensor("wq", [DM, DG], F32, kind="ExternalInput")
    wks_d = nc.dram_tensor("wks", [DM, DG], F32, kind="ExternalInput")
    wkc_d = nc.dram_tensor("wkc", [DM, DG], F32, kind="ExternalInput")
    wvs_d = nc.dram_tensor("wvs", [DM, DG], F32, kind="ExternalInput")
    wvc_d = nc.dram_tensor("wvc", [DM, DG], F32, kind="ExternalInput")
    wo_d = nc.dram_tensor("wo", [DG, DM], F32, kind="ExternalInput")
    bq_d = nc.dram_tensor("bq", [DG], F32, kind="ExternalInput")
    bks_d = nc.dram_tensor("bks", [DG], F32, kind="ExternalInput")
    bkc_d = nc.dram_tensor("bkc", [DG], F32, kind="ExternalInput")
    bvs_d = nc.dram_tensor("bvs", [128, DG], F32, kind="ExternalInput")
    bvc_d = nc.dram_tensor("bvc", [128, DG], F32, kind="ExternalInput")

    attn_o = nc.dram_tensor("attn_o", [HG, SL, KL], F32R, kind="ExternalOutput")
    pout_o = nc.dram_tensor("pout_o", [SL, DM], F32, kind="ExternalOutput")
    tattn_o = nc.dram_tensor("tattn_o", [SL, CL], F32, kind="ExternalOutput")

    with tile.TileContext(nc) as tc, ExitStack() as ctx:
        # ---- persistent pools ----
        pw = ctx.enter_context(tc.tile_pool(name="pw", bufs=1))
        pqkv = ctx.enter_context(tc.tile_pool(name="pqkv", bufs=1))

        ident = pw.tile([128, 128], F32, name="ident")
        make_identity(nc, ident[:])
        ident_r = pw.tile([128, 128], F32R, name="identr")
        nc.vector.tensor_copy(ident_r[:], ident[:])

        wor = pw.tile([64, 4, DM], F32R, name="wor")
        mctx_f = pw.tile([128, CL], F32, name="mctxf")
        nc.sync.dma_start(mctx_f[:], mctx_d[:])
        mctx_t = pw.tile([128, CL], F32R, name="mctx")
        nc.vector.tensor_copy(mctx_t[:], mctx_f[:])

        # biases: [DG] -> [128, 2] (d-chunk j at col j)
        def load_bias2(dram):
            t = pw.tile([128, 2], F32, name=f"b_{dram.name}")
            nc.sync.dma_start(t[:], dram.ap().rearrange("(j p) -> p j", p=128))
            return t

        bq_t = load_bias2(bq_d)
        bks_t = load_bias2(bks_d)
        bkc_t = load_bias2(bkc_d)

        bvsb = pw.tile([128, DG], F32, name="bvsb")
        bvcb = pw.tile([128, DG], F32, name="bvcb")
        nc.sync.dma_start(bvsb[:], bvs_d[:])
        nc.sync.dma_start(bvcb[:], bvc_d[:])

        msk_t = pqkv.tile([128, 8, SL], U8, name="msk")
        nc.sync.dma_start(msk_t[:], msk_d.ap().rearrange("(i p) k -> p i k", p=128))

        qt = [pqkv.tile([128, SL], F32R, name=f"qt{d}") for d in range(2)]
        kt = [pqkv.tile([128, KL], F32R, name=f"kt{d}") for d in range(2)]
        v_t = pqkv.tile([128, 12, DG], F32R, name="v")

        # ---- phase A+B: transposes + projections (scoped pools) ----
        with ExitStack() as pctx:
            pwt = pctx.enter_context(tc.tile_pool(name="pwt", bufs=1))
            psm = pctx.enter_context(tc.tile_pool(name="psm", bufs=1))
            pxc = pctx.enter_context(tc.tile_pool(name="pxc", bufs=4))
            pxt = pctx.enter_context(tc.tile_pool(name="pxt", bufs=1))
            tpsum = pctx.enter_context(tc.tile_pool(name="tpsum", bufs=2, space="PSUM"))
            ppsum = pctx.enter_context(tc.tile_pool(name="ppsum", bufs=3, space="PSUM"))

            def stage_w(dram):
                f = pwt.tile([128, 8, DG], F32, name="wf")
                nc.sync.dma_start(f[:], dram.ap().rearrange("(j p) d -> p j d", p=128))
                r = pwt.tile([128, 8, DG], F32R, name="wr")
                nc.vector.tensor_copy(r[:], f[:])
                return r

            # wo: [DG, DM] -> [64, 4, DM] f32r
            wof = pwt.tile([64, 4, DM], F32, name="wf")
            nc.sync.dma_start(wof[:], wo_d.ap().rearrange("(j p) n -> p j n", p=64))
            nc.vector.tensor_copy(wor[:], wof[:])

            # raw X, C tiles [tok, f] + transposes, grouped by 4 tok-tiles
            xt = [pxt.tile([128, SL], F32R, name=f"xt{j}") for j in range(8)]
            ct = [pxt.tile([128, CL], F32R, name=f"ct{j}") for j in range(8)]
            for m in range(2):
                xr = []
                for l in range(4):
                    t = pxc.tile([128, DM], F32, name="x")
                    nc.sync.dma_start(t[:], x_d[bass.ts(m * 4 + l, 128), :])
                    xr.append(t)
                for j in range(8):
                    pt = tpsum.tile([128, 512], F32, name="tp")
                    for l in range(4):
                        nc.tensor.matmul(pt[:, bass.ts(l, 128)],
                                         xr[l][:, bass.ts(j, 128)], ident[:],
                                         is_transpose=True,
                                         start=(l == 0), stop=(l == 3))
                    nc.scalar.copy(xt[j][:, bass.ts(m, 512)], pt[:])
            cr = []
            for l in range(4):
                t = pxc.tile([128, DM], F32, name="x")
                nc.sync.dma_start(t[:], c_d[bass.ts(l, 128), :])
                cr.append(t)
            for j in range(8):
                pt = tpsum.tile([128, 512], F32, name="tp")
                for l in range(4):
                    nc.tensor.matmul(pt[:, bass.ts(l, 128)],
                                     cr[l][:, bass.ts(j, 128)], ident[:],
                                     is_transpose=True,
                                     start=(l == 0), stop=(l == 3))
                nc.scalar.copy(ct[j][:], pt[:])

            # Q projection
            wqr = stage_w(wq_d)
            for dg in range(2):
                for nh in range(2):
                    ps = ppsum.tile([128, 512], F32, name="pp")
                    for j in range(8):
                        nc.tensor.matmul(ps[:], wqr[:, j, bass.ts(dg, 128)],
                                         xt[j][:, bass.ts(nh, 512)],
                                         start=(j == 0), stop=(j == 7))
                    nc.scalar.activation(qt[dg][:, bass.ts(nh, 512)], ps[:],
                                         AF.Identity, bias=bq_t[:, dg:dg + 1])
            # K self
            wksr = stage_w(wks_d)
            for dg in range(2):
                for nh in range(2):
                    ps = ppsum.tile([128, 512], F32, name="pp")
                    for j in range(8):
                        nc.tensor.matmul(ps[:], wksr[:, j, bass.ts(dg, 128)],
                                         xt[j][:, bass.ts(nh, 512)],
                                         start=(j == 0), stop=(j == 7))
                    nc.scalar.activation(kt[dg][:, bass.ts(nh, 512)], ps[:],
                                         AF.Identity, bias=bks_t[:, dg:dg + 1])
            # K ctx
            wkcr = stage_w(wkc_d)
            for dg in range(2):
                ps = ppsum.tile([128, 512], F32, name="pp")
                for j in range(8):
                    nc.tensor.matmul(ps[:], wkcr[:, j, bass.ts(dg, 128)],
                                     ct[j][:], start=(j == 0), stop=(j == 7))
                nc.scalar.activation(kt[dg][:, SL:SL + CL], ps[:, :CL],
                                     AF.Identity, bias=bkc_t[:, dg:dg + 1])
            # V self
            wvsr = stage_w(wvs_d)
            for ktile in range(8):
                ps = ppsum.tile([128, 512], F32, name="pp")
                for j in range(8):
                    nc.tensor.matmul(ps[:, :DG], xt[j][:, bass.ts(ktile, 128)],
                                     wvsr[:, j, :], start=(j == 0), stop=(j == 7))
                nc.vector.tensor_add(v_t[:, ktile, :], ps[:, :DG], bvsb[:])
            # V ctx
            wvcr = stage_w(wvc_d)
            for ktile in range(8, 12):
                ps = ppsum.tile([128, 512], F32, name="pp")
                for j in range(8):
                    nc.tensor.matmul(ps[:, :DG], ct[j][:, bass.ts(ktile - 8, 128)],
                                     wvcr[:, j, :], start=(j == 0), stop=(j == 7))
                nc.vector.tensor_add(v_t[:, ktile, :], ps[:, :DG], bvcb[:])

        # ---- phase C: attention ----
        ctxT = [pqkv.tile([64, SL], F32R, name=f"ctxTh{h}") for h in range(HG)]

        with ExitStack() as actx:
            pm = actx.enter_context(tc.tile_pool(name="pm", bufs=4))
            pe_ = actx.enter_context(tc.tile_pool(name="pe", bufs=4))
            pat = actx.enter_context(tc.tile_pool(name="pat", bufs=7))
            patt = actx.enter_context(tc.tile_pool(name="patt", bufs=3))
            ptt = actx.enter_context(tc.tile_pool(name="ptt", bufs=2))
            psml = actx.enter_context(tc.tile_pool(name="psml", bufs=4))
            ps_s = actx.enter_context(tc.tile_pool(name="ps_s", bufs=1, space="PSUM"))
            ps_t = actx.enter_context(tc.tile_pool(name="ps_t", bufs=3, space="PSUM"))
            ps_c = actx.enter_context(tc.tile_pool(name="ps_c", bufs=2, space="PSUM"))

            for qg in range(2):
                mbig = []
                for qt_i in range(4):
                    m = pm.tile([128, SL], F32R, name="mbig")
                    nc.vector.tensor_scalar_mul(
                        m[:], msk_t[:, qg * 4 + qt_i, :], NEG)
                    mbig.append(m)

                for h in range(HG):
                    dg, hh = h // 2, h % 2
                    attn_tiles = []
                    for qt_i in range(4):
                        qa = qg * 512 + qt_i * 128
                        # scores + mask (mask injected via identity-matmul)
                        ps1 = ps_s.tile([128, KL], F32, name="ps1")
                        for n in range(3):
                            nc.tensor.matmul(
                                ps1[:, bass.ts(n, 512)],
                                qt[dg][bass.ds(hh * 64, 64), bass.ds(qa, 128)],
                                kt[dg][bass.ds(hh * 64, 64), bass.ts(n, 512)],
                                start=True, stop=False)
                        for n in range(2):
                            nc.tensor.matmul(
                                ps1[:, bass.ts(n, 512)], ident_r[:],
                                mbig[qt_i][:, bass.ts(n, 512)],
                                start=False, stop=False)
                        nc.tensor.matmul(
                            ps1[:, SL:], ident_r[:], mctx_t[:],
                            start=False, stop=True)
                        # exp + row sums (reads psum)
                        e_sb = pe_.tile([128, KL], F32, name="e")
                        sums = psml.tile([128, 1], F32, name="sums")
                        nc.scalar.activation(e_sb[:], ps1[:], AF.Exp,
                                             accum_out=sums[:])
                        recip = psml.tile([128, 1], F32, name="recip")
                        nc.vector.reciprocal(recip[:], sums[:])
                        a_sb = pat.tile([128, KL], F32R, name="a")
                        nc.vector.tensor_scalar_mul(a_sb[:], e_sb[:], recip[:])
                        nc.sync.dma_start(attn_o[h, bass.ds(qa, 128), :], a_sb[:])
                        attn_tiles.append(a_sb)
                        if h == 0:
                            sumc = psml.tile([128, 1], F32, name="sumc")
                            nc.vector.tensor_reduce(sumc[:], e_sb[:, SL:],
                                                    axis=mybir.AxisListType.X,
                                                    op=mybir.AluOpType.add)
                            recc = psml.tile([128, 1], F32, name="recc")
                            nc.vector.reciprocal(recc[:], sumc[:])
                            ta = ptt.tile([128, CL], F32, name="ta")
                            nc.vector.tensor_scalar_mul(ta[:], e_sb[:, SL:],
                                                         recc[:])
                            nc.sync.dma_start(
                                tattn_o[bass.ds(qa, 128), :], ta[:])

                    # transpose attn -> attnT [k, q512], then context mm,
                    # interleaved per ktile (3 rotating attnT slots)
                    ps_ctx = ps_c.tile([64, 512], F32, name="pc")
                    for ktile in range(12):
                        ptp = ps_t.tile([128, 512], F32R, name="tp2")
                        for qt_i in range(4):
                            nc.tensor.matmul(
                                ptp[:, bass.ts(qt_i, 128)],
                                attn_tiles[qt_i][:, bass.ts(ktile, 128)],
                                ident_r[:], is_transpose=True,
                                start=(qt_i == 0), stop=(qt_i == 3))
                        at = patt.tile([128, 512], F32R, name="at")
                        if ktile % 2 == 0:
                            nc.vector.tensor_copy(at[:], ptp[:])
                        else:
                            nc.scalar.copy(at[:], ptp[:])
                        nc.tensor.matmul(
                            ps_ctx[:],
                            v_t[:, ktile, bass.ds(h * 64, 64)],
                            at[:],
                            start=(ktile == 0), stop=(ktile == 11))
                    nc.scalar.copy(ctxT[h][:, bass.ts(qg, 512)], ps_ctx[:])

        # ---- phase D: output projection ----
        with ExitStack() as octx:
            po = octx.enter_context(tc.tile_pool(name="po", bufs=3))
            ps_o = octx.enter_context(tc.tile_pool(name="ps_o", bufs=4, space="PSUM"))
            for tt in range(8):
                ot = po.tile([128, DM], F32, name="ot")
                for nh in range(2):
                    ps = ps_o.tile([128, 512], F32, name="pso")
                    for p in range(HG):
                        nc.tensor.matmul(ps[:], ctxT[p][:, bass.ts(tt, 128)],
                                         wor[:, p, bass.ts(nh, 512)],
                                         start=(p == 0), stop=(p == HG - 1))
                    nc.scalar.copy(ot[:, bass.ts(nh, 512)], ps[:])
                nc.sync.dma_start(pout_o[bass.ts(tt, 128), :], ot[:])

    nc.compile()
    return nc


def _prep(inputs):
    """Host-side slicing: returns per-core in_maps."""
    f32 = np.float32
    x = np.ascontiguousarray(np.asarray(inputs["self_kvq"], f32))
    c = np.ascontiguousarray(np.asarray(inputs["ctx_kv"], f32))
    smask = np.asarray(inputs["self_mask"]).astype(np.uint8)
    cmask = np.asarray(inputs["ctx_mask"]).astype(f32)
    ctx_bias = float(np.asarray(inputs["ctx_bias"]).reshape(-1)[0])
    scale = 1.0 / math.sqrt(D)

    in_maps = []
    for core in range(8):
        b, g = core // 4, core % 4
        gs, ge = g * DG, (g + 1) * DG
        mctx = np.broadcast_to(
            (cmask[b, 0] * NEG + ctx_bias).astype(f32)[None, :], (128, CL))
        m = {
            "x": x[b], "c": c[b],
            "msk": np.ascontiguousarray(smask[b]),
            "mctx": np.ascontiguousarray(mctx),
            "wq": np.ascontiguousarray(
                (np.asarray(inputs["Wq"], f32)[gs:ge] * scale).T),
            "bq": np.ascontiguousarray(
                np.asarray(inputs["bq"], f32)[gs:ge] * scale),
            "wks": np.ascontiguousarray(np.asarray(inputs["Wks"], f32)[gs:ge].T),
            "bks": np.ascontiguousarray(np.asarray(inputs["bks"], f32)[gs:ge]),
            "wkc": np.ascontiguousarray(np.asarray(inputs["Wkc"], f32)[gs:ge].T),
            "bkc": np.ascontiguousarray(np.asarray(inputs["bkc"], f32)[gs:ge]),
            "wvs": np.ascontiguousarray(np.asarray(inputs["Wvs"], f32)[gs:ge].T),
            "wvc": np.ascontiguousarray(np.asarray(inputs["Wvc"], f32)[gs:ge].T),
            "bvs": np.ascontiguousarray(np.broadcast_to(
                np.asarray(inputs["bvs"], f32)[None, gs:ge], (128, DG))),
            "bvc": np.ascontiguousarray(np.broadcast_to(
                np.asarray(inputs["bvc"], f32)[None, gs:ge], (128, DG))),
            "wo": np.ascontiguousarray(np.asarray(inputs["Wo"], f32)[:, gs:ge].T),
        }
        in_maps.append(m)
    return in_maps


def kernel(trace=False, trace_cores=None, **inputs):
    if "nc" not in _CACHE:
        _CACHE["nc"] = _build()
    nc = _CACHE["nc"]
    in_maps = _prep(inputs)
    def _run():
        try:
            return run_bass_kernel_spmd(nc, in_maps, core_ids=list(range(8)),
                                        trace=trace, trace_cores=trace_cores)
        except ModuleNotFoundError:
            return run_bass_kernel_spmd(nc, in_maps, core_ids=list(range(8)))

    try:
        res = _run()
    except Exception as e:   # transient NRT_EXEC_UNIT_UNRECOVERABLE wedge
        if "UNRECOVERABLE" not in str(e) and "UNAVAILABLE" not in str(e):
            raise
        res = _run()
    _CACHE["last"] = res

    bo = np.asarray(inputs["bo"], np.float32)
    attn = np.empty((B, H, SL, KL), np.float32)
    out = np.empty((B, SL, DM), np.float32)
    top = np.empty((B, SL, CL), np.float32)
    for b in range(B):
        acc = None
        for g in range(4):
            r = res.results[b * 4 + g]
            attn[b, 4 * g:4 * g + 4] = r["attn_o"]
            acc = r["pout_o"] if acc is None else acc + r["pout_o"]
        out[b] = acc + bo
        top[b] = res.results[b * 4]["tattn_o"]
    return out, top, attn
